# Initial kernel scaffold
#
"""Trainium2 Bass kernel for the fused cross-head attention block.

Problem shapes (hardcoded):
  x_c, x_t: [8, 256, 128, 128] f32; Wq/Wk/Wv/Wo: [256, 256]; biases [256].
  out: [8, 256, 128, 128] f32.

Math per sample (C=256, nh=8, hd=32, N=H*W=16384 tokens):
  x = x_c + x_t                                    (channel-major [C, N])
  q/k/v = per-token linear projections
  per token: dot[h,g] = q_h . k_g * hd^-0.5  (8x8 gram across heads)
             attn = softmax_g(dot);  o_h = sum_g attn[h,g] v_g
  out = Wo @ o (+ bo)                              (channel-major [C, N])

Sharding: pure data parallel, one sample per NeuronCore (8 cores).

Kernel design (per core, per 128-token tile):
  - PE pass 1 with x-block as the *stationary* operand and a precomputed
    [256, 768] concat(WqT', WkT', WvT) as the moving operand: q,k,v come out
    token-major in PSUM. q,k use head-major permuted output channels
    (h*32+c); v keeps the natural (c,g) interleaved order so the numerator
    multiply has unit-stride innermost g.
  - one ACT copy PSUM->SBUF downcasts qkv to bf16.
  - gram: DVE bf16 multiply with broadcast APs [p, h, g, c] + bf16 add-tree
    over c; exp on ACT (scale folded); denominator reduce + reciprocal and
    attn-normalization on DVE; numerator: bf16 multiply [p, h, c, g] +
    add-tree over g -> o' token-major (head-major channels).
  - PE transpose of o' (bf16) + out-projection with WoT stationary ->
    fp32 PSUM, DMA'd straight to DRAM channel-major.
"""

import sys

if "/opt/trn_rl_repo" not in sys.path:
    sys.path.insert(0, "/opt/trn_rl_repo")

from contextlib import ExitStack

import ml_dtypes
import numpy as np

import concourse.bass as bass
import concourse.tile as tile
from concourse import mybir
from concourse.bass_utils import run_bass_kernel_spmd
from concourse.masks import make_identity

B, C, HH, WW = 8, 256, 128, 128
NH, HD = 8, 32
N = HH * WW  # 16384 tokens per sample
TT = 128  # tokens per tile
SCALE = float(HD) ** -0.5

F32 = mybir.dt.float32
F32R = mybir.dt.float32r
BF16 = mybir.dt.bfloat16


def build_kernel(n_tiles=N // TT, has_qkv_bias=False, has_o_bias=False):
    nc = bass.Bass(trn_type="TRN2")

    xc = nc.declare_dram_parameter("xc", [C, N], F32, isOutput=False)
    xt = nc.declare_dram_parameter("xt", [C, N], F32, isOutput=False)
    # [256, 768] f32: columns 0:256 WqT' (head-major rows), 256:512 WkT',
    # 512:768 WvT (natural order)
    wqkv = nc.declare_dram_parameter("wqkv", [C, 3 * C], F32, isOutput=False)
    # [256, 256] bf16: Wo^T (input channels head-major == natural merge order)
    wot = nc.declare_dram_parameter("wot", [C, C], BF16, isOutput=False)
    bqkv = nc.declare_dram_parameter("bqkv", [1, 3 * C], F32, isOutput=False)
    bo = nc.declare_dram_parameter("bo", [1, C], F32, isOutput=False)
    out = nc.declare_dram_parameter("out", [C, N], F32, isOutput=True)

    with ExitStack() as ctx, tile.TileContext(nc) as tc:
        singles = ctx.enter_context(tc.tile_pool(name="singles", bufs=1))
        xs_pool = ctx.enter_context(tc.tile_pool(name="xs", bufs=3))
        qkv_pool = ctx.enter_context(tc.tile_pool(name="qkv", bufs=2))
        att_pool = ctx.enter_context(tc.tile_pool(name="att", bufs=2))
        sm_pool = ctx.enter_context(tc.tile_pool(name="sm", bufs=2))
        psum_qkv = ctx.enter_context(tc.tile_pool(name="ps_qkv", bufs=2, space="PSUM"))
        psum_tr = ctx.enter_context(tc.tile_pool(name="ps_tr", bufs=2, space="PSUM"))
        psum_out = ctx.enter_context(tc.tile_pool(name="ps_out", bufs=2, space="PSUM"))

        # resident constants
        wqkv_sb = singles.tile([128, 2, 3 * C], F32)  # [k-block, 2, 768]
        nc.sync.dma_start(out=wqkv_sb, in_=wqkv.rearrange("(b p) m -> p b m", b=2))
        wot_sb = singles.tile([128, 2, C], BF16)
        nc.sync.dma_start(out=wot_sb, in_=wot.rearrange("(b p) m -> p b m", b=2))
        ident = singles.tile([128, 128], BF16)
        make_identity(nc, ident)
        if has_qkv_bias:
            bqkv_sb = singles.tile([1, 3 * C], F32)
            nc.sync.dma_start(out=bqkv_sb, in_=bqkv)
            ones_tok = singles.tile([1, TT], F32)
            nc.vector.memset(ones_tok, 1.0)
        if has_o_bias:
            bo_sb = singles.tile([1, C], F32)
            nc.sync.dma_start(out=bo_sb, in_=bo)
            ones_tok_o = singles.tile([1, TT], BF16)
            nc.vector.memset(ones_tok_o, 1.0)

        xc_r = xc.rearrange("(b p) n -> b p n", b=2)
        xt_r = xt.rearrange("(b p) n -> b p n", b=2)
        out_r = out.rearrange("(b p) n -> b p n", b=2)

        for i in range(n_tiles):
            tok = slice(i * TT, (i + 1) * TT)
            # ---- load x_c, x_t (channel-major, 2 partition blocks each)
            xc_t = xs_pool.tile([128, 2, TT], F32, tag="xc")
            xt_t = xs_pool.tile([128, 2, TT], F32, tag="xt")
            for b in range(2):
                nc.sync.dma_start(out=xc_t[:, b], in_=xc_r[b, :, tok])
                nc.sync.dma_start(out=xt_t[:, b], in_=xt_r[b, :, tok])
            xs_t = xs_pool.tile([128, 2, TT], F32, tag="xs")
            nc.vector.tensor_add(xs_t, xc_t, xt_t)

            # ---- PE pass 1: q,k,v token-major in PSUM [tok, 768]
            qkv_ps = psum_qkv.tile([TT, 3 * C], F32)
            for b in range(2):
                nc.tensor.matmul(
                    qkv_ps,
                    lhsT=xs_t[:, b].bitcast(F32R),
                    rhs=wqkv_sb[:, b].bitcast(F32R),
                    start=(b == 0),
                    stop=(b == 1) and not has_qkv_bias,
                )
            if has_qkv_bias:
                nc.tensor.matmul(
                    qkv_ps,
                    lhsT=ones_tok,
                    rhs=bqkv_sb,
                    start=False,
                    stop=True,
                )

            # ---- downcast to bf16 in SBUF (one ACT copy)
            qkv_sb = qkv_pool.tile([TT, 3 * C], BF16)
            nc.scalar.copy(out=qkv_sb, in_=qkv_ps)
            q = qkv_sb[:, 0:C]  # [tok, (h,c)] head-major
            k = qkv_sb[:, C : 2 * C]  # [tok, (g,c)] head-major
            v = qkv_sb[:, 2 * C : 3 * C]  # [tok, (c,g)] natural

            # ---- gram: P[p, h, g, c] = q[p,h,c] * k[p,g,c]
            gp = att_pool.tile([TT, NH, NH, HD], BF16, tag="gramp")
            q_b = q.rearrange("p (h c) -> p h c", h=NH).unsqueeze(2).broadcast_to(
                [TT, NH, NH, HD]
            )
            k_b = k.rearrange("p (g c) -> p g c", g=NH).unsqueeze(1).broadcast_to(
                [TT, NH, NH, HD]
            )
            nc.vector.tensor_mul(gp, q_b, k_b)
            # add-tree over c: 32 -> 16 -> 8 -> 4 -> 2 -> 1
            t16 = att_pool.tile([TT, NH, NH, 16], BF16, tag="t16")
            nc.vector.tensor_add(t16, gp[:, :, :, 0:16], gp[:, :, :, 16:32])
            t8 = att_pool.tile([TT, NH, NH, 8], BF16, tag="t8")
            nc.vector.tensor_add(t8, t16[:, :, :, 0:8], t16[:, :, :, 8:16])
            t4 = att_pool.tile([TT, NH, NH, 4], BF16, tag="t4")
            nc.vector.tensor_add(t4, t8[:, :, :, 0:4], t8[:, :, :, 4:8])
            t2 = att_pool.tile([TT, NH, NH, 2], BF16, tag="t2")
            nc.vector.tensor_add(t2, t4[:, :, :, 0:2], t4[:, :, :, 2:4])
            dot = att_pool.tile([TT, NH, NH], BF16, tag="dot")
            nc.vector.tensor_add(dot, t2[:, :, :, 0], t2[:, :, :, 1])

            # ---- softmax over g (no max-subtraction; logits are tiny)
            ex = sm_pool.tile([TT, NH, NH], BF16, tag="ex")
            nc.scalar.activation(
                out=ex, in_=dot, func=mybir.ActivationFunctionType.Exp, scale=SCALE
            )
            den = sm_pool.tile([TT, NH], F32, tag="den")
            nc.vector.reduce_sum(den, ex, axis=mybir.AxisListType.X)
            rec = sm_pool.tile([TT, NH], F32, tag="rec")
            nc.vector.reciprocal(rec, den)
            recb = sm_pool.tile([TT, NH], BF16, tag="recb")
            nc.vector.tensor_copy(recb, rec)
            attn = sm_pool.tile([TT, NH, NH], BF16, tag="attn")
            nc.vector.tensor_mul(
                attn, ex, recb.unsqueeze(2).broadcast_to([TT, NH, NH])
            )

            # ---- numerator: PN[p, h, c, g] = attn[p,h,g] * v[p,c,g]
            pn = att_pool.tile([TT, NH, HD, NH], BF16, tag="nump")
            a_b = attn.unsqueeze(2).broadcast_to([TT, NH, HD, NH])
            v_b = v.rearrange("p (c g) -> p c g", g=NH).unsqueeze(1).broadcast_to(
                [TT, NH, HD, NH]
            )
            nc.vector.tensor_mul(pn, a_b, v_b)
            # add-tree over g: 8 -> 4 -> 2 -> 1
            n4 = att_pool.tile([TT, NH, HD, 4], BF16, tag="n4")
            nc.vector.tensor_add(n4, pn[:, :, :, 0:4], pn[:, :, :, 4:8])
            n2 = att_pool.tile([TT, NH, HD, 2], BF16, tag="n2")
            nc.vector.tensor_add(n2, n4[:, :, :, 0:2], n4[:, :, :, 2:4])
            oprime = att_pool.tile([TT, C], BF16, tag="oprime")
            o_v = oprime.rearrange("p (h c) -> p h c", h=NH)
            nc.vector.tensor_add(o_v, n2[:, :, :, 0], n2[:, :, :, 1])

            # ---- transpose o' -> channel-major (PE), copy to SBUF
            otr_ps = psum_tr.tile([128, 2, TT], F32)
            for b in range(2):
                nc.tensor.transpose(
                    otr_ps[:, b], oprime[:, b * 128 : (b + 1) * 128], ident
                )
            otr_sb = qkv_pool.tile([128, 2, TT], BF16, tag="otr")
            nc.scalar.copy(out=otr_sb, in_=otr_ps)

            # ---- out projection: out[c_out, tok] = sum_ci WoT[ci, c_out] o'T[ci, tok]
            out_ps = psum_out.tile([128, 2, TT], F32)
            for m in range(2):
                for b in range(2):
                    nc.tensor.matmul(
                        out_ps[:, m],
                        lhsT=wot_sb[:, b, m * 128 : (m + 1) * 128],
                        rhs=otr_sb[:, b],
                        start=(b == 0),
                        stop=(b == 1) and not has_o_bias,
                    )
                if has_o_bias:
                    nc.tensor.matmul(
                        out_ps[:, m],
                        lhsT=bo_sb[:, m * 128 : (m + 1) * 128].bitcast(BF16),
                        rhs=ones_tok_o,
                        start=False,
                        stop=True,
                    )
            for b in range(2):
                nc.sync.dma_start(out=out_r[b, :, tok], in_=out_ps[:, b])

    return nc


_PERM = np.array([c * NH + h for h in range(NH) for c in range(HD)])


def _prep_weights(Wq, bq, Wk, bk, Wv, bv, Wo, bo):
    wq_p = Wq[_PERM]
    wk_p = Wk[_PERM]
    wqkv = np.concatenate([wq_p.T, wk_p.T, Wv.T], axis=1).astype(np.float32)
    wqkv = np.ascontiguousarray(wqkv)
    wot = np.ascontiguousarray(Wo.T).astype(ml_dtypes.bfloat16)
    bqkv = np.concatenate([bq[_PERM], bk[_PERM], bv]).astype(np.float32)[None]
    bo_a = bo.astype(np.float32)[None]
    return wqkv, wot, bqkv, bo_a


def kernel(x_c, x_t, Wq, bq, Wk, bk, Wv, bv, Wo, bo):
    x_c = np.asarray(x_c, dtype=np.float32)
    x_t = np.asarray(x_t, dtype=np.float32)
    wqkv, wot, bqkv, bo_a = _prep_weights(
        np.asarray(Wq, np.float32),
        np.asarray(bq, np.float32),
        np.asarray(Wk, np.float32),
        np.asarray(bk, np.float32),
        np.asarray(Wv, np.float32),
        np.asarray(bv, np.float32),
        np.asarray(Wo, np.float32),
        np.asarray(bo, np.float32),
    )
    has_qkv_bias = bool(np.any(bqkv))
    has_o_bias = bool(np.any(bo_a))
    nc = build_kernel(has_qkv_bias=has_qkv_bias, has_o_bias=has_o_bias)

    in_maps = []
    for b in range(B):
        in_maps.append(
            {
                "xc": np.ascontiguousarray(x_c[b].reshape(C, N)),
                "xt": np.ascontiguousarray(x_t[b].reshape(C, N)),
                "wqkv": wqkv,
                "wot": wot,
                "bqkv": bqkv,
                "bo": bo_a,
            }
        )
    res = run_bass_kernel_spmd(nc, in_maps, list(range(B)))
    outs = []
    for b in range(B):
        outs.append(np.asarray(res.results[b]["out"]).reshape(C, HH, WW))
    return np.stack(outs).astype(np.float32)


if __name__ == "__main__":
    rng = np.random.default_rng(0)
    ins = {
        "x_c": rng.standard_normal((B, C, HH, WW), dtype=np.float32),
        "x_t": rng.standard_normal((B, C, HH, WW), dtype=np.float32),
        "Wq": (rng.standard_normal((C, C)) * 0.02).astype(np.float32),
        "bq": np.zeros(C, np.float32),
        "Wk": (rng.standard_normal((C, C)) * 0.02).astype(np.float32),
        "bk": np.zeros(C, np.float32),
        "Wv": (rng.standard_normal((C, C)) * 0.02).astype(np.float32),
        "bv": np.zeros(C, np.float32),
        "Wo": (rng.standard_normal((C, C)) * 0.02).astype(np.float32),
        "bo": np.zeros(C, np.float32),
    }
    out = kernel(**ins)
    print(out.shape, out.dtype)


# revision 12
# speedup vs baseline: 1.1068x; 1.1068x over previous
"""Trainium2 Bass kernel for the fused cross-head attention block.

Problem shapes (hardcoded):
  x_c, x_t: [8, 256, 128, 128] f32; Wq/Wk/Wv/Wo: [256, 256]; biases [256].
  out: [8, 256, 128, 128] f32.

Math per sample (C=256, nh=8, hd=32, N=H*W=16384 tokens):
  x = x_c + x_t                                    (channel-major [C, N])
  q/k/v = per-token linear projections
  per token: dot[h,g] = q_h . k_g * hd^-0.5  (8x8 gram across heads)
             attn = softmax_g(dot);  o_h = sum_g attn[h,g] v_g
  out = Wo @ o (+ bo)                              (channel-major [C, N])

Sharding: pure data parallel, one sample per NeuronCore (8 cores).

Kernel design (per core, per 128-token tile):
  - PE pass 1 with x-block as the *stationary* operand and a precomputed
    [256, 768] concat(WqT', WkT', WvT) as the moving operand: q,k,v come out
    token-major in PSUM. q,k use head-major permuted output channels
    (h*32+c); v keeps the natural (c,g) interleaved order so the numerator
    multiply has unit-stride innermost g.
  - one ACT copy PSUM->SBUF downcasts qkv to bf16.
  - gram: DVE bf16 multiply with broadcast APs [p, h, g, c] + bf16 add-tree
    over c; exp on ACT (scale folded); denominator reduce + reciprocal and
    attn-normalization on DVE; numerator: bf16 multiply [p, h, c, g] +
    add-tree over g -> o' token-major (head-major channels).
  - PE transpose of o' (bf16) + out-projection with WoT stationary ->
    fp32 PSUM, DMA'd straight to DRAM channel-major.
"""

import os
import sys

if "/opt/trn_rl_repo" not in sys.path:
    sys.path.insert(0, "/opt/trn_rl_repo")

from contextlib import ExitStack

import ml_dtypes
import numpy as np

import concourse.bass as bass
import concourse.bacc as bacc
import concourse.tile as tile
from concourse import mybir
from concourse.bass_utils import run_bass_kernel_spmd
from concourse.masks import make_identity

B, C, HH, WW = 8, 256, 128, 128
NH, HD = 8, 32
N = HH * WW  # 16384 tokens per sample
TT = 128  # tokens per tile
SCALE = float(HD) ** -0.5

F32 = mybir.dt.float32
F32R = mybir.dt.float32r
BF16 = mybir.dt.bfloat16
F16 = mybir.dt.float16


def build_kernel(n_tiles=N // TT, has_qkv_bias=False, has_o_bias=False):
    nc = bacc.Bacc(trn_type="TRN2")

    xc = nc.declare_dram_parameter("xc", [C, N], F32, isOutput=False)
    xt = nc.declare_dram_parameter("xt", [C, N], F32, isOutput=False)
    # [256, 768] f32: columns 0:256 WqT' (head-major rows), 256:512 WkT',
    # 512:768 WvT (natural order)
    wqkv = nc.declare_dram_parameter("wqkv", [C, 3 * C], F16, isOutput=False)
    # [256, 256] bf16: Wo^T (input channels head-major == natural merge order)
    wot = nc.declare_dram_parameter("wot", [C, C], F16, isOutput=False)
    bqkv = nc.declare_dram_parameter("bqkv", [1, 3 * C], F16, isOutput=False)
    bo = nc.declare_dram_parameter("bo", [1, C], F32, isOutput=False)
    out = nc.declare_dram_parameter("out", [C, N], F32, isOutput=True)

    with tile.TileContext(nc) as tc, ExitStack() as ctx:
        singles = ctx.enter_context(tc.tile_pool(name="singles", bufs=1))
        xs_pool = ctx.enter_context(tc.tile_pool(name="xs", bufs=3))
        qkv_pool = ctx.enter_context(tc.tile_pool(name="qkv", bufs=2))
        att_pool = ctx.enter_context(tc.tile_pool(name="att", bufs=2))
        sm_pool = ctx.enter_context(tc.tile_pool(name="sm", bufs=2))
        psum_qkv = ctx.enter_context(tc.tile_pool(name="ps_qkv", bufs=2, space="PSUM"))
        psum_tr = ctx.enter_context(tc.tile_pool(name="ps_tr", bufs=2, space="PSUM"))
        psum_out = ctx.enter_context(tc.tile_pool(name="ps_out", bufs=2, space="PSUM"))

        # resident constants
        wqkv_sb = singles.tile([128, 2, 3 * C], F16)  # [k-block, 2, 768]
        nc.sync.dma_start(out=wqkv_sb, in_=wqkv.rearrange("(b p) m -> p b m", b=2))
        wot_sb = singles.tile([128, 2, C], F16)
        nc.sync.dma_start(out=wot_sb, in_=wot.rearrange("(b p) m -> p b m", b=2))
        ident = singles.tile([128, 128], F16)
        make_identity(nc, ident)
        if has_qkv_bias:
            bqkv_sb = singles.tile([1, 3 * C], F16)
            nc.sync.dma_start(out=bqkv_sb, in_=bqkv)
            ones_tok = singles.tile([1, TT], F16)
            nc.vector.memset(ones_tok, 1.0)
        if has_o_bias:
            bo_sb = singles.tile([1, C], F32)
            nc.sync.dma_start(out=bo_sb, in_=bo)
            ones_tok_o = singles.tile([1, TT], F16)
            nc.vector.memset(ones_tok_o, 1.0)

        xc_r = xc.rearrange("(b p) n -> p b n", b=2)
        xt_r = xt.rearrange("(b p) n -> p b n", b=2)
        out_r = out.rearrange("(b p) n -> b p n", b=2)

        for i in range(n_tiles):
            tok = slice(i * TT, (i + 1) * TT)
            # ---- load x_c, x_t (channel-major, 2 partition blocks each)
            xc_t = xs_pool.tile([128, 2, TT], F32, tag="xc")
            xt_t = xs_pool.tile([128, 2, TT], F32, tag="xt")
            xs_t = xs_pool.tile([128, 2, TT], F16, tag="xs")
            for b in range(2):
                nc.sync.dma_start(out=xc_t[:, b], in_=xc_r[:, b, tok])
                nc.sync.dma_start(out=xt_t[:, b], in_=xt_r[:, b, tok])
                nc.vector.tensor_add(xs_t[:, b], xc_t[:, b], xt_t[:, b])

            # ---- PE pass 1: q,k,v token-major in PSUM [tok, 768]
            qkv_ps = psum_qkv.tile([TT, 3 * C], F32)
            for lo, hi in ((0, 512), (512, 768)):
                for b in range(2):
                    nc.tensor.matmul(
                        qkv_ps[:, lo:hi],
                        lhsT=xs_t[:, b],
                        rhs=wqkv_sb[:, b, lo:hi],
                        start=(b == 0),
                        stop=(b == 1) and not has_qkv_bias,
                    )
                if has_qkv_bias:
                    nc.tensor.matmul(
                        qkv_ps[:, lo:hi],
                        lhsT=ones_tok,
                        rhs=bqkv_sb[:, lo:hi],
                        start=False,
                        stop=True,
                    )

            # ---- downcast to bf16 in SBUF (one ACT copy)
            qkv_sb = qkv_pool.tile([TT, 3 * C], F16)
            nc.scalar.copy(out=qkv_sb, in_=qkv_ps)
            q = qkv_sb[:, 0:C]  # [tok, (h,c)] head-major
            k = qkv_sb[:, C : 2 * C]  # [tok, (g,c)] head-major
            v = qkv_sb[:, 2 * C : 3 * C]  # [tok, (c,g)] natural

            # ---- gram: P[p, h, g, c] = q[p,h,c] * k[p,g,c]
            gp = att_pool.tile([TT, NH, NH, HD], F16, tag="gramp")
            q_b = q.rearrange("p (h c) -> p h c", h=NH).unsqueeze(2).broadcast_to(
                [TT, NH, NH, HD]
            )
            k_b = k.rearrange("p (g c) -> p g c", g=NH).unsqueeze(1).broadcast_to(
                [TT, NH, NH, HD]
            )
            nc.vector.tensor_mul(gp, q_b, k_b)
            # add-tree over c: 32 -> 16 -> 8 -> 4 -> 2 -> 1
            t16 = att_pool.tile([TT, NH, NH, 16], F16, tag="t16")
            nc.vector.tensor_add(t16, gp[:, :, :, 0:16], gp[:, :, :, 16:32])
            t8 = att_pool.tile([TT, NH, NH, 8], F16, tag="t8")
            nc.vector.tensor_add(t8, t16[:, :, :, 0:8], t16[:, :, :, 8:16])
            t4 = att_pool.tile([TT, NH, NH, 4], F16, tag="t4")
            nc.vector.tensor_add(t4, t8[:, :, :, 0:4], t8[:, :, :, 4:8])
            t2 = att_pool.tile([TT, NH, NH, 2], F16, tag="t2")
            nc.vector.tensor_add(t2, t4[:, :, :, 0:2], t4[:, :, :, 2:4])
            dot = att_pool.tile([TT, NH, NH], F16, tag="dot")
            nc.vector.tensor_add(dot, t2[:, :, :, 0], t2[:, :, :, 1])

            # ---- softmax over g (no max-subtraction; logits are tiny)
            ex = sm_pool.tile([TT, NH, NH], F16, tag="ex")
            nc.scalar.activation(
                out=ex, in_=dot, func=mybir.ActivationFunctionType.Exp, scale=SCALE
            )
            den = sm_pool.tile([TT, NH], F32, tag="den")
            nc.vector.reduce_sum(den, ex, axis=mybir.AxisListType.X)
            rec = sm_pool.tile([TT, NH], F32, tag="rec")
            nc.vector.reciprocal(rec, den)
            recb = sm_pool.tile([TT, NH], F16, tag="recb")
            nc.vector.tensor_copy(recb, rec)
            attn = sm_pool.tile([TT, NH, NH], F16, tag="attn")
            nc.vector.tensor_mul(
                attn, ex, recb.unsqueeze(2).broadcast_to([TT, NH, NH])
            )

            # ---- numerator: PN[p, h, c, g] = attn[p,h,g] * v[p,c,g]
            pn = att_pool.tile([TT, NH, HD, NH], F16, tag="nump")
            a_b = attn.unsqueeze(2).broadcast_to([TT, NH, HD, NH])
            v_b = v.rearrange("p (c g) -> p c g", g=NH).unsqueeze(1).broadcast_to(
                [TT, NH, HD, NH]
            )
            nc.vector.tensor_mul(pn, a_b, v_b)
            # add-tree over g: 8 -> 4 -> 2 -> 1
            n4 = att_pool.tile([TT, NH, HD, 4], F16, tag="n4")
            nc.vector.tensor_add(n4, pn[:, :, :, 0:4], pn[:, :, :, 4:8])
            n2 = att_pool.tile([TT, NH, HD, 2], F16, tag="n2")
            nc.vector.tensor_add(n2, n4[:, :, :, 0:2], n4[:, :, :, 2:4])
            oprime = att_pool.tile([TT, C], F16, tag="oprime")
            o_v = oprime.rearrange("p (h c) -> p h c", h=NH)
            nc.vector.tensor_add(o_v, n2[:, :, :, 0], n2[:, :, :, 1])

            # ---- transpose o' -> channel-major (PE), copy to SBUF
            otr_ps = psum_tr.tile([128, 2, TT], F16)
            for b in range(2):
                nc.tensor.transpose(
                    otr_ps[:, b], oprime[:, b * 128 : (b + 1) * 128], ident
                )
            otr_sb = qkv_pool.tile([128, 2, TT], F16, tag="otr")
            nc.scalar.copy(out=otr_sb, in_=otr_ps)

            # ---- out projection: out[c_out, tok] = sum_ci WoT[ci, c_out] o'T[ci, tok]
            out_ps = psum_out.tile([128, 2, TT], F32)
            for m in range(2):
                for b in range(2):
                    nc.tensor.matmul(
                        out_ps[:, m],
                        lhsT=wot_sb[:, b, m * 128 : (m + 1) * 128],
                        rhs=otr_sb[:, b],
                        start=(b == 0),
                        stop=(b == 1) and not has_o_bias,
                    )
                if has_o_bias:
                    nc.tensor.matmul(
                        out_ps[:, m],
                        lhsT=bo_sb[:, m * 128 : (m + 1) * 128].bitcast(F16),
                        rhs=ones_tok_o,
                        start=False,
                        stop=True,
                    )
            out_sb = qkv_pool.tile([128, 2, TT], F32, tag="outsb")
            nc.scalar.copy(out=out_sb, in_=out_ps)
            for b in range(2):
                nc.sync.dma_start(out=out_r[b, :, tok], in_=out_sb[:, b])

    nc.compile()
    return nc


_PERM = np.array([c * NH + h for h in range(NH) for c in range(HD)])


def _prep_weights(Wq, bq, Wk, bk, Wv, bv, Wo, bo):
    wq_p = Wq[_PERM]
    wk_p = Wk[_PERM]
    wqkv = np.concatenate([wq_p.T, wk_p.T, Wv.T], axis=1).astype(np.float32)
    wqkv = np.ascontiguousarray(wqkv).astype(np.float16)
    wot = np.ascontiguousarray(Wo.T).astype(np.float16)
    bqkv = np.concatenate([bq[_PERM], bk[_PERM], bv]).astype(np.float16)[None]
    bo_a = bo.astype(np.float32)[None]
    return wqkv, wot, bqkv, bo_a


def kernel(x_c, x_t, Wq, bq, Wk, bk, Wv, bv, Wo, bo):
    x_c = np.asarray(x_c, dtype=np.float32)
    x_t = np.asarray(x_t, dtype=np.float32)
    wqkv, wot, bqkv, bo_a = _prep_weights(
        np.asarray(Wq, np.float32),
        np.asarray(bq, np.float32),
        np.asarray(Wk, np.float32),
        np.asarray(bk, np.float32),
        np.asarray(Wv, np.float32),
        np.asarray(bv, np.float32),
        np.asarray(Wo, np.float32),
        np.asarray(bo, np.float32),
    )
    has_qkv_bias = bool(np.any(bqkv))
    has_o_bias = bool(np.any(bo_a))
    nc = build_kernel(has_qkv_bias=has_qkv_bias, has_o_bias=has_o_bias)

    in_maps = []
    for b in range(B):
        in_maps.append(
            {
                "xc": np.ascontiguousarray(x_c[b].reshape(C, N)),
                "xt": np.ascontiguousarray(x_t[b].reshape(C, N)),
                "wqkv": wqkv,
                "wot": wot,
                "bqkv": bqkv,
                "bo": bo_a,
            }
        )
    res = run_bass_kernel_spmd(nc, in_maps, list(range(B)))
    outs = []
    for b in range(B):
        outs.append(np.asarray(res.results[b]["out"]).reshape(C, HH, WW))
    return np.stack(outs).astype(np.float32)


def _install_ntff_shim():
    """Recreate the missing antenv.axon_hooks module + ctypes NTFF hook
    (mirrors trn_agent_boot.trn_boot's degraded-silently path)."""
    import contextlib
    import ctypes
    import types

    try:
        from antenv.axon_hooks import get_axon_ntff_profile_hook  # noqa: F401

        return True
    except ImportError:
        pass
    import antenv

    mod = types.ModuleType("antenv.axon_hooks")
    mod._hook = None

    def set_axon_ntff_profile_hook(h):
        mod._hook = h

    def get_axon_ntff_profile_hook():
        return mod._hook

    mod.set_axon_ntff_profile_hook = set_axon_ntff_profile_hook
    mod.get_axon_ntff_profile_hook = get_axon_ntff_profile_hook
    sys.modules["antenv.axon_hooks"] = mod
    antenv.axon_hooks = mod

    so_path = "/opt/axon/libaxon_pjrt.so"
    if not os.path.exists(so_path):
        return False
    lib = ctypes.CDLL(so_path)
    if not hasattr(lib, "axon_start_nrt_profile"):
        return False
    lib.axon_start_nrt_profile.argtypes = [
        ctypes.POINTER(ctypes.c_int64),
        ctypes.c_size_t,
    ]
    lib.axon_start_nrt_profile.restype = ctypes.c_int64
    lib.axon_stop_nrt_profile.argtypes = [ctypes.c_char_p]
    lib.axon_stop_nrt_profile.restype = ctypes.c_int64

    @contextlib.contextmanager
    def _hook(output_dir, device_ids):
        import jax

        jax.devices()
        if device_ids:
            ids = (ctypes.c_int64 * len(device_ids))(*device_ids)
            rc = lib.axon_start_nrt_profile(ids, len(device_ids))
        else:
            rc = lib.axon_start_nrt_profile(None, 0)
        if rc != 0:
            raise RuntimeError(f"axon_start_nrt_profile rc={rc}")
        try:
            yield
        finally:
            n = lib.axon_stop_nrt_profile(str(output_dir).encode())
            print(f"profile: {n} file(s) written to {output_dir}")

    set_axon_ntff_profile_hook(_hook)
    return True


def profile_run(inputs_np):
    """Run once more with NTFF tracing on core 0; return exec_time_ns."""
    import concourse.bass_utils as bu

    _install_ntff_shim()
    bu.upload_artifacts = lambda d: "local://" + d  # no S3 in this container
    x_c = np.asarray(inputs_np["x_c"], np.float32)
    x_t = np.asarray(inputs_np["x_t"], np.float32)
    wqkv, wot, bqkv, bo_a = _prep_weights(
        *[
            np.asarray(inputs_np[k], np.float32)
            for k in ("Wq", "bq", "Wk", "bk", "Wv", "bv", "Wo", "bo")
        ]
    )
    nc = build_kernel(
        has_qkv_bias=bool(np.any(bqkv)), has_o_bias=bool(np.any(bo_a))
    )
    in_maps = []
    for b in range(B):
        in_maps.append(
            {
                "xc": np.ascontiguousarray(x_c[b].reshape(C, N)),
                "xt": np.ascontiguousarray(x_t[b].reshape(C, N)),
                "wqkv": wqkv,
                "wot": wot,
                "bqkv": bqkv,
                "bo": bo_a,
            }
        )
    res = run_bass_kernel_spmd(nc, in_maps, list(range(B)), trace=True)
    return res.exec_time_ns


if __name__ == "__main__":
    rng = np.random.default_rng(0)
    ins = {
        "x_c": rng.standard_normal((B, C, HH, WW), dtype=np.float32),
        "x_t": rng.standard_normal((B, C, HH, WW), dtype=np.float32),
        "Wq": (rng.standard_normal((C, C)) * 0.02).astype(np.float32),
        "bq": np.zeros(C, np.float32),
        "Wk": (rng.standard_normal((C, C)) * 0.02).astype(np.float32),
        "bk": np.zeros(C, np.float32),
        "Wv": (rng.standard_normal((C, C)) * 0.02).astype(np.float32),
        "bv": np.zeros(C, np.float32),
        "Wo": (rng.standard_normal((C, C)) * 0.02).astype(np.float32),
        "bo": np.zeros(C, np.float32),
    }
    out = kernel(**ins)
    print(out.shape, out.dtype)
